# revision 16
# baseline (speedup 1.0000x reference)
"""Multi-head attention Trainium2 kernel (8 NeuronCores).

Sharding: core c owns batch b = c//2 and heads h0 = (c%2)*6 .. h0+6 (tensor
parallel over heads x data parallel over batch). Each core computes its 6
heads' attention and a partial output projection; the host sums the two
partial projections per batch element and adds the output bias.

Per-core layout (all matmuls in float32r, fp32 PSUM accumulation):
  xt  [D=768, S=2048]   x[b] transposed on host, D on partitions
  Qt/Kt [384, 2048]     (h e) on partitions, computed as Wq^T @ x^T
  V   [S, 576]          keys on partitions; per head pair: [V_a|ones|V_b]
  scores^T [keys, q]    per 128-key chunk, via lhsT=Kt slice (K=64)
  exp on ScalarE PSUM->SBUF, fused 1/8 scale
  ctx^T += [V_h|ones]^T @ exp: 64 psum rows of unnormalized ctx +
                        64 rows of replicated softmax denominator
  normalize on VectorE, project with Wo chunks, DMA partial out.
"""
import sys

sys.path.insert(0, "/opt/trn_rl_repo")

from contextlib import ExitStack

import numpy as np

import concourse.bacc as bacc
import concourse.bass as bass
import concourse.mybir as mybir
import concourse.tile as tile
from concourse.bass_utils import run_bass_kernel_spmd

f32 = mybir.dt.float32
f32r = mybir.dt.float32r
AF = mybir.ActivationFunctionType
ALU = mybir.AluOpType

B, S, D = 4, 2048, 768
H, E = 12, 64
HL = 6              # heads per core
F = HL * E          # 384: local concat-head feature dim
ND = D // 128       # 6 contraction chunks over D
NF = F // 128       # 3 chunks over F
NK = S // 128       # 16 key chunks
QB = 512            # q block (matmul moving free dim)
NQB = S // QB       # 4
KG = 2              # key chunks per exp group
VW = NF * 192       # V tile width: 3 pairs x [V_a|ones|V_b]
NCORES = 8

_NC = None


def _build():
    nc = bacc.Bacc()
    xt_d = nc.declare_dram_parameter("xt", [D, S], f32, isOutput=False)
    wq_d = nc.declare_dram_parameter("wq", [D, F], f32, isOutput=False)
    wk_d = nc.declare_dram_parameter("wk", [D, F], f32, isOutput=False)
    wv_d = nc.declare_dram_parameter("wv", [D, F], f32, isOutput=False)
    wo_d = nc.declare_dram_parameter("wo", [F, D], f32, isOutput=False)
    bq_d = nc.declare_dram_parameter("bq", [F, 1], f32, isOutput=False)
    bk_d = nc.declare_dram_parameter("bk", [F, 1], f32, isOutput=False)
    bv_d = nc.declare_dram_parameter("bv", [1, F], f32, isOutput=False)
    ones_d = nc.declare_dram_parameter("ones", [1, NF * E], f32, isOutput=False)
    y_d = nc.declare_dram_parameter("y", [S, D], f32, isOutput=True)

    with tile.TileContext(nc) as tc, ExitStack() as ctx:
        big = ctx.enter_context(tc.tile_pool(name="big", bufs=12))
        vpool = ctx.enter_context(tc.tile_pool(name="vpool", bufs=NK))
        wpool = ctx.enter_context(tc.tile_pool(name="wpool", bufs=18))
        wopool = ctx.enter_context(tc.tile_pool(name="wopool", bufs=NF))
        epool = ctx.enter_context(tc.tile_pool(name="epool", bufs=3))
        opool = ctx.enter_context(tc.tile_pool(name="opool", bufs=3))
        npool = ctx.enter_context(tc.tile_pool(name="npool", bufs=2))
        cpool = ctx.enter_context(tc.tile_pool(name="cpool", bufs=1))
        psA = ctx.enter_context(tc.tile_pool(name="psA", bufs=4, space="PSUM"))
        pssc = ctx.enter_context(tc.tile_pool(name="pssc", bufs=2, space="PSUM"))

        # --- constant/bias tiles ---
        bq_sb = cpool.tile([128, NF], f32, name="bq_sb", tag="bq")
        nc.sync.dma_start(out=bq_sb, in_=bq_d.rearrange("(m p) o -> p m o", p=128))
        bk_sb = cpool.tile([128, NF], f32, name="bk_sb", tag="bk")
        nc.sync.dma_start(out=bk_sb, in_=bk_d.rearrange("(m p) o -> p m o", p=128))
        # bv broadcast across partitions via 0-stride DRAM read
        bv_bc = cpool.tile([128, F], f32, name="bv_bc", tag="bv")
        bv_src = bv_d[0:1, :]
        bv_ap = bass.AP(tensor=bv_src.tensor, offset=bv_src.offset,
                        ap=[[0, 128]] + list(bv_src.ap)[1:])
        nc.sync.dma_start(out=bv_bc, in_=bv_ap)

        # --- HAM warmer operands: tiny bf16 tiles (values irrelevant) ---
        bf16 = mybir.dt.bfloat16
        dmw = cpool.tile([128, 64], bf16, name="dmw", tag="dmw")
        dmx = cpool.tile([128, 64], bf16, name="dmx", tag="dmx")
        ones_bc_src = ones_d[0:1, 0:64]
        ones_bc = bass.AP(tensor=ones_bc_src.tensor, offset=ones_bc_src.offset,
                          ap=[[0, 128]] + list(ones_bc_src.ap)[1:])
        nc.gpsimd.dma_start(out=dmw, in_=ones_bc)
        nc.gpsimd.dma_start(out=dmx, in_=ones_bc)

        # --- input/weight tiles: gpsimd DMA casts f32 -> f32r on the fly ---
        xt_t = []
        for kd in range(ND):
            t = big.tile([128, S], f32r, tag="big", name=f"xt{kd}")
            nc.gpsimd.dma_start(out=t, in_=xt_d[kd * 128:(kd + 1) * 128, :])
            xt_t.append(t)
        wq_t, wk_t, wv_t = [], [], []
        for nm, dd, lst in (("wq", wq_d, wq_t), ("wk", wk_d, wk_t), ("wv", wv_d, wv_t)):
            for kd in range(ND):
                t = wpool.tile([128, F], f32r, tag="w", name=f"{nm}{kd}")
                nc.gpsimd.dma_start(out=t, in_=dd[kd * 128:(kd + 1) * 128, :])
                lst.append(t)
        wo_t = []
        for kf in range(NF):
            t = wopool.tile([128, D], f32r, tag="wo", name=f"wo{kf}")
            nc.gpsimd.dma_start(out=t, in_=wo_d[kf * 128:(kf + 1) * 128, :])
            wo_t.append(t)

        # --- phase 1a: Qt, Kt [F, S] ---
        qt_t = [big.tile([128, S], f32r, tag="big", name=f"qt{m}") for m in range(NF)]
        kt_t = [big.tile([128, S], f32r, tag="big", name=f"kt{m}") for m in range(NF)]
        for w_t, out_t, b_sb in ((wq_t, qt_t, bq_sb), (wk_t, kt_t, bk_sb)):
            for m in range(NF):
                for nq in range(NQB):
                    ps = psA.tile([128, QB], f32, tag="bank", name=f"p1_{m}_{nq}")
                    for kd in range(ND):
                        nc.tensor.matmul(
                            ps[:, :],
                            lhsT=w_t[kd][:, m * 128:(m + 1) * 128],
                            rhs=xt_t[kd][:, nq * QB:(nq + 1) * QB],
                            start=(kd == 0), stop=(kd == ND - 1),
                        )
                    nc.vector.tensor_scalar_add(
                        out_t[m][:, nq * QB:(nq + 1) * QB], ps[:, :],
                        b_sb[:, m:m + 1],
                    )

        # --- phase 1b: V [S, VW], per pair [V_a | ones | V_b] ---
        ones_src = ones_d[0:1, :].rearrange("o (pair e) -> o pair e", e=E)
        ones_ap = bass.AP(tensor=ones_src.tensor, offset=ones_src.offset,
                          ap=[[0, 128]] + list(ones_src.ap)[1:])
        v_t = []
        for mk in range(NK):
            t = vpool.tile([128, VW], f32r, tag="v", name=f"v{mk}")
            t3o = t[:].rearrange("p (pair c) -> p pair c", c=192)
            nc.gpsimd.dma_start(out=t3o[:, :, E:2 * E], in_=ones_ap)
            ps = psA.tile([128, F], f32, tag="bank", name=f"p1v_{mk}", padded_shape=[128, QB])
            for kd in range(ND):
                nc.tensor.matmul(
                    ps[:, :],
                    lhsT=xt_t[kd][:, mk * 128:(mk + 1) * 128],
                    rhs=wv_t[kd][:, :],
                    start=(kd == 0), stop=(kd == ND - 1),
                )
            # psum cols: (pair, hh, e); dest pair block: [V_a | ones | V_b]
            ps3 = ps.rearrange("p (pair hh e) -> p pair hh e", hh=2, e=E)
            bv3 = bv_bc.rearrange("p (pair hh e) -> p pair hh e", hh=2, e=E)
            t3 = t[:].rearrange("p (pair c) -> p pair c", c=192)
            nc.vector.tensor_tensor(
                t3[:, :, 0:E], ps3[:, :, 0, :], bv3[:, :, 0, :], op=ALU.add)
            nc.vector.tensor_tensor(
                t3[:, :, 2 * E:3 * E], ps3[:, :, 1, :], bv3[:, :, 1, :], op=ALU.add)
            v_t.append(t)

        # --- phase 2: attention ---
        ctxt_t = [big.tile([128, S], f32r, tag="big", name=f"ctxt{m}") for m in range(NF)]
        # bf16 matmuls keep the PE HAM activity monitor warm (2.4 GHz);
        # a pure fp32r stream gets throttled to 1.2 GHz. All warmer MMs
        # write the same psum tile (PE-order WAW, no sems needed).
        dummy_ps = psA.tile([128, QB], f32, tag="bank", name="dummy_ps")
        for hp in range(NF):          # head pair (partitions 0:64 / 64:128)
            for nq in range(NQB):
                cps = [
                    psA.tile([128, QB], f32, tag="bank", name=f"c{hp}_{nq}_{hh}")
                    for hh in range(2)
                ]
                for g in range(NK // KG):
                    for hh in range(2):
                        hsl = slice(hh * E, (hh + 1) * E)
                        sps = pssc.tile([128, KG * QB], f32, tag="sc",
                                        name=f"s{hp}_{nq}_{g}_{hh}")
                        for j in range(KG):
                            mk = g * KG + j
                            nc.tensor.matmul(
                                sps[:, j * QB:(j + 1) * QB],
                                lhsT=kt_t[hp][hsl, mk * 128:(mk + 1) * 128],
                                rhs=qt_t[hp][hsl, nq * QB:(nq + 1) * QB],
                                start=True, stop=True,
                            )
                        esb = epool.tile([128, KG * QB], f32r, tag="e",
                                         name=f"e{hp}_{nq}_{g}_{hh}")
                        nc.scalar.activation(esb[:], sps[:], AF.Exp, scale=0.125)
                        for j in range(KG):
                            mk = g * KG + j
                            base = hp * 192 + hh * E
                            nc.tensor.matmul(
                                cps[hh][:, :],
                                lhsT=v_t[mk][:, base:base + 128],
                                rhs=esb[:, j * QB:(j + 1) * QB],
                                start=(g == 0 and j == 0),
                                stop=(g == NK // KG - 1 and j == KG - 1),
                            )
                        nc.tensor.matmul(
                            dummy_ps[0:64, 0:64], lhsT=dmw[:, :], rhs=dmx[:, :],
                            start=True, stop=True, skip_group_check=True,
                        )
                for hh in range(2):
                    # head a: ctx rows 0:64, sums 64:128; head b swapped
                    ctx_sl = slice(0, E) if hh == 0 else slice(E, 128)
                    sum_sl = slice(E, 128) if hh == 0 else slice(0, E)
                    r = npool.tile([E, QB], f32, tag="n", name=f"n{hp}_{nq}_{hh}")
                    nc.vector.reciprocal(r[:], cps[hh][sum_sl, :])
                    nc.vector.tensor_tensor(
                        ctxt_t[hp][hh * E:(hh + 1) * E, nq * QB:(nq + 1) * QB],
                        cps[hh][ctx_sl, :], r[:], op=ALU.mult,
                    )

        # --- phase 3: partial output projection [S, D] ---
        for mq in range(NK):
            ps = pssc.tile([128, D], f32, tag="sc", name=f"o{mq}")
            for kf in range(NF):
                lhsT = ctxt_t[kf][:, mq * 128:(mq + 1) * 128]
                nc.tensor.matmul(ps[:, 0:512], lhsT=lhsT,
                                 rhs=wo_t[kf][:, 0:512],
                                 start=(kf == 0), stop=(kf == NF - 1))
                nc.tensor.matmul(ps[:, 512:D], lhsT=lhsT,
                                 rhs=wo_t[kf][:, 512:D],
                                 start=(kf == 0), stop=(kf == NF - 1))
            nc.tensor.matmul(
                dummy_ps[0:64, 0:64], lhsT=dmw[:, :], rhs=dmx[:, :],
                start=True, stop=True, skip_group_check=True,
            )
            osb = opool.tile([128, D], f32, tag="o", name=f"ot{mq}")
            nc.vector.tensor_copy(osb[:, 0:512], ps[:, 0:512])
            nc.vector.tensor_copy(osb[:, 512:D], ps[:, 512:D])
            nc.sync.dma_start(out=y_d[mq * 128:(mq + 1) * 128, :], in_=osb[:])
    nc.compile()
    return nc


def _get_nc():
    global _NC
    if _NC is None:
        _NC = _build()
    return _NC


def kernel(x, Wq, bq, Wk, bk, Wv, bv, Wo, bo, _trace=False):
    x = np.asarray(x, dtype=np.float32)
    Wq = np.asarray(Wq, dtype=np.float32)
    bq = np.asarray(bq, dtype=np.float32)
    Wk = np.asarray(Wk, dtype=np.float32)
    bk = np.asarray(bk, dtype=np.float32)
    Wv = np.asarray(Wv, dtype=np.float32)
    bv = np.asarray(bv, dtype=np.float32)
    Wo = np.asarray(Wo, dtype=np.float32)
    bo = np.asarray(bo, dtype=np.float32)

    nc = _get_nc()
    in_maps = []
    for c in range(NCORES):
        b = c // 2
        h0 = (c % 2) * HL
        in_maps.append({
            "xt": np.ascontiguousarray(x[b].T),
            "wq": np.ascontiguousarray(Wq[h0:h0 + HL].transpose(1, 0, 2).reshape(D, F)),
            "wk": np.ascontiguousarray(Wk[h0:h0 + HL].transpose(1, 0, 2).reshape(D, F)),
            "wv": np.ascontiguousarray(Wv[h0:h0 + HL].transpose(1, 0, 2).reshape(D, F)),
            "wo": np.ascontiguousarray(Wo[h0 * E:(h0 + HL) * E]),
            "bq": np.ascontiguousarray(bq[h0:h0 + HL].reshape(F, 1)),
            "bk": np.ascontiguousarray(bk[h0:h0 + HL].reshape(F, 1)),
            "bv": np.ascontiguousarray(bv[h0:h0 + HL].reshape(1, F)),
            "ones": np.ones((1, NF * E), np.float32),
        })
    res = run_bass_kernel_spmd(nc, in_maps, list(range(NCORES)), trace=_trace)
    out = np.empty((B, S, D), np.float32)
    for b in range(B):
        out[b] = res.results[2 * b]["y"] + res.results[2 * b + 1]["y"] + bo[None, :]
    if _trace:
        kernel.last_exec_time_ns = res.exec_time_ns
        kernel.last_results = res
    return out


# revision 17
# speedup vs baseline: 1.4380x; 1.4380x over previous
"""Multi-head attention Trainium2 kernel (8 NeuronCores).

Sharding: core c owns batch b = c//2 and heads h0 = (c%2)*6 .. h0+6 (tensor
parallel over heads x data parallel over batch). Each core computes its 6
heads' attention and a partial output projection; the host sums the two
partial projections per batch element and adds the output bias.

Per-core layout (all matmuls in float32r, fp32 PSUM accumulation):
  xt  [D=768, S=2048]   x[b] transposed on host, D on partitions
  Qt/Kt [384, 2048]     (h e) on partitions, computed as Wq^T @ x^T
  V   [S, 576]          keys on partitions; per head pair: [V_a|ones|V_b]
  scores^T [keys, q]    per 128-key chunk, via lhsT=Kt slice (K=64)
  exp on ScalarE PSUM->SBUF, fused 1/8 scale
  ctx^T += [V_h|ones]^T @ exp: 64 psum rows of unnormalized ctx +
                        64 rows of replicated softmax denominator
  normalize on VectorE, project with Wo chunks, DMA partial out.
"""
import sys

sys.path.insert(0, "/opt/trn_rl_repo")

from contextlib import ExitStack

import numpy as np

import concourse.bacc as bacc
import concourse.bass as bass
import concourse.mybir as mybir
import concourse.tile as tile
from concourse.bass_utils import run_bass_kernel_spmd

f32 = mybir.dt.float32
f32r = mybir.dt.float32r
AF = mybir.ActivationFunctionType
ALU = mybir.AluOpType

B, S, D = 4, 2048, 768
H, E = 12, 64
HL = 6              # heads per core
F = HL * E          # 384: local concat-head feature dim
ND = D // 128       # 6 contraction chunks over D
NF = F // 128       # 3 chunks over F
NK = S // 128       # 16 key chunks
QB = 512            # q block (matmul moving free dim)
NQB = S // QB       # 4
KG = 2              # key chunks per exp group
VW = NF * 192       # V tile width: 3 pairs x [V_a|ones|V_b]
NCORES = 8

_NC = None


def _build():
    nc = bacc.Bacc()
    xt_d = nc.declare_dram_parameter("xt", [D, S], f32, isOutput=False)
    wq_d = nc.declare_dram_parameter("wq", [D, F], f32, isOutput=False)
    wk_d = nc.declare_dram_parameter("wk", [D, F], f32, isOutput=False)
    wv_d = nc.declare_dram_parameter("wv", [D, F], f32, isOutput=False)
    wo_d = nc.declare_dram_parameter("wo", [F, D], f32, isOutput=False)
    bq_d = nc.declare_dram_parameter("bq", [F, 1], f32, isOutput=False)
    bk_d = nc.declare_dram_parameter("bk", [F, 1], f32, isOutput=False)
    bv_d = nc.declare_dram_parameter("bv", [1, F], f32, isOutput=False)
    ones_d = nc.declare_dram_parameter("ones", [1, NF * E], f32, isOutput=False)
    y_d = nc.declare_dram_parameter("y", [S, D], f32, isOutput=True)

    with tile.TileContext(nc) as tc, ExitStack() as ctx:
        big = ctx.enter_context(tc.tile_pool(name="big", bufs=15))
        vpool = ctx.enter_context(tc.tile_pool(name="vpool", bufs=NK))
        wpool = ctx.enter_context(tc.tile_pool(name="wpool", bufs=12))
        wopool = ctx.enter_context(tc.tile_pool(name="wopool", bufs=NF))
        epool = ctx.enter_context(tc.tile_pool(name="epool", bufs=2))
        opool = ctx.enter_context(tc.tile_pool(name="opool", bufs=2))
        npool = ctx.enter_context(tc.tile_pool(name="npool", bufs=2))
        cpool = ctx.enter_context(tc.tile_pool(name="cpool", bufs=1))
        psA = ctx.enter_context(tc.tile_pool(name="psA", bufs=4, space="PSUM"))
        pssc = ctx.enter_context(tc.tile_pool(name="pssc", bufs=2, space="PSUM"))

        # --- constant/bias tiles ---
        bq_sb = cpool.tile([128, NF], f32, name="bq_sb", tag="bq")
        nc.sync.dma_start(out=bq_sb, in_=bq_d.rearrange("(m p) o -> p m o", p=128))
        bk_sb = cpool.tile([128, NF], f32, name="bk_sb", tag="bk")
        nc.sync.dma_start(out=bk_sb, in_=bk_d.rearrange("(m p) o -> p m o", p=128))
        # bv broadcast across partitions via 0-stride DRAM read
        bv_bc = cpool.tile([128, F], f32, name="bv_bc", tag="bv")
        bv_src = bv_d[0:1, :]
        bv_ap = bass.AP(tensor=bv_src.tensor, offset=bv_src.offset,
                        ap=[[0, 128]] + list(bv_src.ap)[1:])
        nc.sync.dma_start(out=bv_bc, in_=bv_ap)

        # --- input/weight tiles: gpsimd DMA casts f32 -> f32r on the fly ---
        xt_t = []
        for kd in range(ND):
            t = big.tile([128, S], f32r, tag="big", name=f"xt{kd}")
            nc.gpsimd.dma_start(out=t, in_=xt_d[kd * 128:(kd + 1) * 128, :])
            xt_t.append(t)
        wq_t, wk_t, wv_t = [], [], []
        for nm, dd, lst in (("wq", wq_d, wq_t), ("wk", wk_d, wk_t), ("wv", wv_d, wv_t)):
            for kd in range(ND):
                t = wpool.tile([128, F], f32r, tag="w", name=f"{nm}{kd}")
                nc.gpsimd.dma_start(out=t, in_=dd[kd * 128:(kd + 1) * 128, :])
                lst.append(t)
        wo_t = []
        for kf in range(NF):
            t = wopool.tile([128, D], f32r, tag="wo", name=f"wo{kf}")
            nc.gpsimd.dma_start(out=t, in_=wo_d[kf * 128:(kf + 1) * 128, :])
            wo_t.append(t)

        # --- phase 1a: Qt (zero-padded per head, K=128 scores), Kt paired ---
        # qt_t[2*hp+hh]: head's Q rows in its pair-partition slots, zeros in
        # the other 64 partitions, so the score matmul can contract over the
        # full 128 partitions of paired Kt (full HAM duty, no row tiling).
        qt_t = [big.tile([128, S], f32r, tag="big", name=f"qt{h}") for h in range(2 * NF)]
        kt_t = [big.tile([128, S], f32r, tag="big", name=f"kt{m}") for m in range(NF)]
        for m in range(NF):
            for nq in range(NQB):
                sl = slice(nq * QB, (nq + 1) * QB)
                ps = psA.tile([128, QB], f32, tag="bank", name=f"p1q_{m}_{nq}")
                for kd in range(ND):
                    nc.tensor.matmul(
                        ps[:, :],
                        lhsT=wq_t[kd][:, m * 128:(m + 1) * 128],
                        rhs=xt_t[kd][:, nq * QB:(nq + 1) * QB],
                        start=(kd == 0), stop=(kd == ND - 1),
                    )
                qa, qb = qt_t[2 * m], qt_t[2 * m + 1]
                nc.vector.tensor_scalar_add(qa[0:E, sl], ps[0:E, :], bq_sb[0:E, m:m + 1])
                nc.vector.tensor_scalar_mul(qa[E:128, sl], ps[E:128, :], 0.0)
                nc.vector.tensor_scalar_mul(qb[0:E, sl], ps[0:E, :], 0.0)
                nc.vector.tensor_scalar_add(qb[E:128, sl], ps[E:128, :], bq_sb[E:128, m:m + 1])
        for m in range(NF):
            for nq in range(NQB):
                ps = psA.tile([128, QB], f32, tag="bank", name=f"p1k_{m}_{nq}")
                for kd in range(ND):
                    nc.tensor.matmul(
                        ps[:, :],
                        lhsT=wk_t[kd][:, m * 128:(m + 1) * 128],
                        rhs=xt_t[kd][:, nq * QB:(nq + 1) * QB],
                        start=(kd == 0), stop=(kd == ND - 1),
                    )
                nc.vector.tensor_scalar_add(
                    kt_t[m][:, nq * QB:(nq + 1) * QB], ps[:, :],
                    bk_sb[:, m:m + 1],
                )

        # --- phase 1b: V [S, VW], per pair [V_a | ones | V_b] ---
        ones_src = ones_d[0:1, :].rearrange("o (pair e) -> o pair e", e=E)
        ones_ap = bass.AP(tensor=ones_src.tensor, offset=ones_src.offset,
                          ap=[[0, 128]] + list(ones_src.ap)[1:])
        v_t = []
        for mk in range(NK):
            t = vpool.tile([128, VW], f32r, tag="v", name=f"v{mk}")
            t3o = t[:].rearrange("p (pair c) -> p pair c", c=192)
            nc.gpsimd.dma_start(out=t3o[:, :, E:2 * E], in_=ones_ap)
            ps = psA.tile([128, F], f32, tag="bank", name=f"p1v_{mk}", padded_shape=[128, QB])
            for kd in range(ND):
                nc.tensor.matmul(
                    ps[:, :],
                    lhsT=xt_t[kd][:, mk * 128:(mk + 1) * 128],
                    rhs=wv_t[kd][:, :],
                    start=(kd == 0), stop=(kd == ND - 1),
                )
            # psum cols: (pair, hh, e); dest pair block: [V_a | ones | V_b]
            ps3 = ps.rearrange("p (pair hh e) -> p pair hh e", hh=2, e=E)
            bv3 = bv_bc.rearrange("p (pair hh e) -> p pair hh e", hh=2, e=E)
            t3 = t[:].rearrange("p (pair c) -> p pair c", c=192)
            nc.vector.tensor_tensor(
                t3[:, :, 0:E], ps3[:, :, 0, :], bv3[:, :, 0, :], op=ALU.add)
            nc.vector.tensor_tensor(
                t3[:, :, 2 * E:3 * E], ps3[:, :, 1, :], bv3[:, :, 1, :], op=ALU.add)
            v_t.append(t)

        # --- phase 2: attention ---
        ctxt_t = [big.tile([128, S], f32r, tag="big", name=f"ctxt{m}") for m in range(NF)]
        for hp in range(NF):          # head pair (partitions 0:64 / 64:128)
            for nq in range(NQB):
                cps = [
                    psA.tile([128, QB], f32, tag="bank", name=f"c{hp}_{nq}_{hh}")
                    for hh in range(2)
                ]
                for g in range(NK // KG):
                    for hh in range(2):
                        sps = pssc.tile([128, KG * QB], f32, tag="sc",
                                        name=f"s{hp}_{nq}_{g}_{hh}")
                        for j in range(KG):
                            mk = g * KG + j
                            nc.tensor.matmul(
                                sps[:, j * QB:(j + 1) * QB],
                                lhsT=kt_t[hp][:, mk * 128:(mk + 1) * 128],
                                rhs=qt_t[2 * hp + hh][:, nq * QB:(nq + 1) * QB],
                                start=True, stop=True,
                            )
                        esb = epool.tile([128, KG * QB], f32r, tag="e",
                                         name=f"e{hp}_{nq}_{g}_{hh}")
                        nc.scalar.activation(esb[:], sps[:], AF.Exp, scale=0.125)
                        for j in range(KG):
                            mk = g * KG + j
                            base = hp * 192 + hh * E
                            nc.tensor.matmul(
                                cps[hh][:, :],
                                lhsT=v_t[mk][:, base:base + 128],
                                rhs=esb[:, j * QB:(j + 1) * QB],
                                start=(g == 0 and j == 0),
                                stop=(g == NK // KG - 1 and j == KG - 1),
                            )
                for hh in range(2):
                    # head a: ctx rows 0:64, sums 64:128; head b swapped
                    ctx_sl = slice(0, E) if hh == 0 else slice(E, 128)
                    sum_sl = slice(E, 128) if hh == 0 else slice(0, E)
                    r = npool.tile([E, QB], f32, tag="n", name=f"n{hp}_{nq}_{hh}")
                    nc.vector.reciprocal(r[:], cps[hh][sum_sl, :])
                    nc.vector.tensor_tensor(
                        ctxt_t[hp][hh * E:(hh + 1) * E, nq * QB:(nq + 1) * QB],
                        cps[hh][ctx_sl, :], r[:], op=ALU.mult,
                    )

        # --- phase 3: partial output projection [S, D] ---
        for mq in range(NK):
            ps = pssc.tile([128, D], f32, tag="sc", name=f"o{mq}")
            for kf in range(NF):
                lhsT = ctxt_t[kf][:, mq * 128:(mq + 1) * 128]
                nc.tensor.matmul(ps[:, 0:512], lhsT=lhsT,
                                 rhs=wo_t[kf][:, 0:512],
                                 start=(kf == 0), stop=(kf == NF - 1))
                nc.tensor.matmul(ps[:, 512:D], lhsT=lhsT,
                                 rhs=wo_t[kf][:, 512:D],
                                 start=(kf == 0), stop=(kf == NF - 1))
            osb = opool.tile([128, D], f32, tag="o", name=f"ot{mq}")
            nc.vector.tensor_copy(osb[:, 0:512], ps[:, 0:512])
            nc.vector.tensor_copy(osb[:, 512:D], ps[:, 512:D])
            nc.sync.dma_start(out=y_d[mq * 128:(mq + 1) * 128, :], in_=osb[:])
    nc.compile()
    return nc


def _get_nc():
    global _NC
    if _NC is None:
        _NC = _build()
    return _NC


def kernel(x, Wq, bq, Wk, bk, Wv, bv, Wo, bo, _trace=False):
    x = np.asarray(x, dtype=np.float32)
    Wq = np.asarray(Wq, dtype=np.float32)
    bq = np.asarray(bq, dtype=np.float32)
    Wk = np.asarray(Wk, dtype=np.float32)
    bk = np.asarray(bk, dtype=np.float32)
    Wv = np.asarray(Wv, dtype=np.float32)
    bv = np.asarray(bv, dtype=np.float32)
    Wo = np.asarray(Wo, dtype=np.float32)
    bo = np.asarray(bo, dtype=np.float32)

    nc = _get_nc()
    in_maps = []
    for c in range(NCORES):
        b = c // 2
        h0 = (c % 2) * HL
        in_maps.append({
            "xt": np.ascontiguousarray(x[b].T),
            "wq": np.ascontiguousarray(Wq[h0:h0 + HL].transpose(1, 0, 2).reshape(D, F)),
            "wk": np.ascontiguousarray(Wk[h0:h0 + HL].transpose(1, 0, 2).reshape(D, F)),
            "wv": np.ascontiguousarray(Wv[h0:h0 + HL].transpose(1, 0, 2).reshape(D, F)),
            "wo": np.ascontiguousarray(Wo[h0 * E:(h0 + HL) * E]),
            "bq": np.ascontiguousarray(bq[h0:h0 + HL].reshape(F, 1)),
            "bk": np.ascontiguousarray(bk[h0:h0 + HL].reshape(F, 1)),
            "bv": np.ascontiguousarray(bv[h0:h0 + HL].reshape(1, F)),
            "ones": np.ones((1, NF * E), np.float32),
        })
    res = run_bass_kernel_spmd(nc, in_maps, list(range(NCORES)), trace=_trace)
    out = np.empty((B, S, D), np.float32)
    for b in range(B):
        out[b] = res.results[2 * b]["y"] + res.results[2 * b + 1]["y"] + bo[None, :]
    if _trace:
        kernel.last_exec_time_ns = res.exec_time_ns
        kernel.last_results = res
    return out


# revision 18
# speedup vs baseline: 1.5813x; 1.0997x over previous
"""Multi-head attention Trainium2 kernel (8 NeuronCores).

Sharding: core c owns batch b = c//2 and heads h0 = (c%2)*6 .. h0+6 (tensor
parallel over heads x data parallel over batch). Each core computes its 6
heads' attention and a partial output projection; the host sums the two
partial projections per batch element and adds the output bias.

Per-core layout (all matmuls in float32r, fp32 PSUM accumulation):
  xt  [D=768, S=2048]   x[b] transposed on host, D on partitions
  Qt/Kt [384, 2048]     (h e) on partitions, computed as Wq^T @ x^T
  V   [S, 576]          keys on partitions; per head pair: [V_a|ones|V_b]
  scores^T [keys, q]    per 128-key chunk, via lhsT=Kt slice (K=64)
  exp on ScalarE PSUM->SBUF, fused 1/8 scale
  ctx^T += [V_h|ones]^T @ exp: 64 psum rows of unnormalized ctx +
                        64 rows of replicated softmax denominator
  normalize on VectorE, project with Wo chunks, DMA partial out.
"""
import sys

sys.path.insert(0, "/opt/trn_rl_repo")

from contextlib import ExitStack

import numpy as np

import concourse.bacc as bacc
import concourse.bass as bass
import concourse.mybir as mybir
import concourse.tile as tile
from concourse.bass_utils import run_bass_kernel_spmd

f32 = mybir.dt.float32
f32r = mybir.dt.float32r
AF = mybir.ActivationFunctionType
ALU = mybir.AluOpType

B, S, D = 4, 2048, 768
H, E = 12, 64
HL = 6              # heads per core
F = HL * E          # 384: local concat-head feature dim
ND = D // 128       # 6 contraction chunks over D
NF = F // 128       # 3 chunks over F
NK = S // 128       # 16 key chunks
QB = 512            # q block (matmul moving free dim)
NQB = S // QB       # 4
KG = 2              # key chunks per exp group
VW = NF * 192       # V tile width: 3 pairs x [V_a|ones|V_b]
NCORES = 8

_NC = None


def _build():
    nc = bacc.Bacc()
    xt_d = nc.declare_dram_parameter("xt", [D, S], f32r, isOutput=False)
    wq_d = nc.declare_dram_parameter("wq", [D, F], f32r, isOutput=False)
    wk_d = nc.declare_dram_parameter("wk", [D, F], f32r, isOutput=False)
    wv_d = nc.declare_dram_parameter("wv", [D, F], f32r, isOutput=False)
    wo_d = nc.declare_dram_parameter("wo", [F, D], f32r, isOutput=False)
    bq_d = nc.declare_dram_parameter("bq", [F, 1], f32, isOutput=False)
    bk_d = nc.declare_dram_parameter("bk", [F, 1], f32, isOutput=False)
    bv_d = nc.declare_dram_parameter("bv", [1, F], f32, isOutput=False)
    ones_d = nc.declare_dram_parameter("ones", [1, NF * E], f32r, isOutput=False)
    y_d = nc.declare_dram_parameter("y", [S, D], f32, isOutput=True)

    with tile.TileContext(nc) as tc, ExitStack() as ctx:
        big = ctx.enter_context(tc.tile_pool(name="big", bufs=15))
        vpool = ctx.enter_context(tc.tile_pool(name="vpool", bufs=NK))
        wpool = ctx.enter_context(tc.tile_pool(name="wpool", bufs=12))
        wopool = ctx.enter_context(tc.tile_pool(name="wopool", bufs=NF))
        epool = ctx.enter_context(tc.tile_pool(name="epool", bufs=2))
        opool = ctx.enter_context(tc.tile_pool(name="opool", bufs=3))
        npool = ctx.enter_context(tc.tile_pool(name="npool", bufs=2))
        cpool = ctx.enter_context(tc.tile_pool(name="cpool", bufs=1))
        psA = ctx.enter_context(tc.tile_pool(name="psA", bufs=4, space="PSUM"))
        pssc = ctx.enter_context(tc.tile_pool(name="pssc", bufs=2, space="PSUM"))

        # --- constant/bias tiles ---
        bq_sb = cpool.tile([128, NF], f32, name="bq_sb", tag="bq")
        nc.sync.dma_start(out=bq_sb, in_=bq_d.rearrange("(m p) o -> p m o", p=128))
        bk_sb = cpool.tile([128, NF], f32, name="bk_sb", tag="bk")
        nc.sync.dma_start(out=bk_sb, in_=bk_d.rearrange("(m p) o -> p m o", p=128))
        # bv broadcast across partitions via 0-stride DRAM read
        bv_bc = cpool.tile([128, F], f32, name="bv_bc", tag="bv")
        bv_src = bv_d[0:1, :]
        bv_ap = bass.AP(tensor=bv_src.tensor, offset=bv_src.offset,
                        ap=[[0, 128]] + list(bv_src.ap)[1:])
        nc.sync.dma_start(out=bv_bc, in_=bv_ap)

        # --- input/weight tiles: gpsimd DMA casts f32 -> f32r on the fly ---
        xt_t = []
        for kd in range(ND):
            t = big.tile([128, S], f32r, tag="big", name=f"xt{kd}")
            nc.sync.dma_start(out=t, in_=xt_d[kd * 128:(kd + 1) * 128, :])
            xt_t.append(t)
        wq_t, wk_t, wv_t = [], [], []
        for nm, dd, lst in (("wq", wq_d, wq_t), ("wk", wk_d, wk_t), ("wv", wv_d, wv_t)):
            for kd in range(ND):
                t = wpool.tile([128, F], f32r, tag="w", name=f"{nm}{kd}")
                nc.scalar.dma_start(out=t, in_=dd[kd * 128:(kd + 1) * 128, :])
                lst.append(t)
        wo_t = []
        for kf in range(NF):
            t = wopool.tile([128, D], f32r, tag="wo", name=f"wo{kf}")
            nc.scalar.dma_start(out=t, in_=wo_d[kf * 128:(kf + 1) * 128, :])
            wo_t.append(t)

        # --- phase 1a: Qt (zero-padded per head, K=128 scores), Kt paired ---
        # qt_t[2*hp+hh]: head's Q rows in its pair-partition slots, zeros in
        # the other 64 partitions, so the score matmul can contract over the
        # full 128 partitions of paired Kt (full HAM duty, no row tiling).
        qt_t = [big.tile([128, S], f32r, tag="big", name=f"qt{h}") for h in range(2 * NF)]
        kt_t = [big.tile([128, S], f32r, tag="big", name=f"kt{m}") for m in range(NF)]
        for m in range(NF):
            nc.vector.tensor_scalar_mul(qt_t[2 * m][E:128, :], xt_t[0][E:128, :], 0.0)
            nc.vector.tensor_scalar_mul(qt_t[2 * m + 1][0:E, :], xt_t[0][0:E, :], 0.0)
        for m in range(NF):
            for nq in range(NQB):
                sl = slice(nq * QB, (nq + 1) * QB)
                ps = psA.tile([128, QB], f32, tag="bank", name=f"p1q_{m}_{nq}")
                for kd in range(ND):
                    nc.tensor.matmul(
                        ps[:, :],
                        lhsT=wq_t[kd][:, m * 128:(m + 1) * 128],
                        rhs=xt_t[kd][:, nq * QB:(nq + 1) * QB],
                        start=(kd == 0), stop=(kd == ND - 1),
                    )
                qa, qb = qt_t[2 * m], qt_t[2 * m + 1]
                nc.vector.tensor_scalar_add(qa[0:E, sl], ps[0:E, :], bq_sb[0:E, m:m + 1])
                nc.vector.tensor_scalar_add(qb[E:128, sl], ps[E:128, :], bq_sb[E:128, m:m + 1])
        for m in range(NF):
            for nq in range(NQB):
                ps = psA.tile([128, QB], f32, tag="bank", name=f"p1k_{m}_{nq}")
                for kd in range(ND):
                    nc.tensor.matmul(
                        ps[:, :],
                        lhsT=wk_t[kd][:, m * 128:(m + 1) * 128],
                        rhs=xt_t[kd][:, nq * QB:(nq + 1) * QB],
                        start=(kd == 0), stop=(kd == ND - 1),
                    )
                nc.vector.tensor_scalar_add(
                    kt_t[m][:, nq * QB:(nq + 1) * QB], ps[:, :],
                    bk_sb[:, m:m + 1],
                )

        # --- phase 1b: V [S, VW], per pair [V_a | ones | V_b] ---
        ones_src = ones_d[0:1, :].rearrange("o (pair e) -> o pair e", e=E)
        ones_ap = bass.AP(tensor=ones_src.tensor, offset=ones_src.offset,
                          ap=[[0, 128]] + list(ones_src.ap)[1:])
        v_t = []
        for mk in range(NK):
            t = vpool.tile([128, VW], f32r, tag="v", name=f"v{mk}")
            t3o = t[:].rearrange("p (pair c) -> p pair c", c=192)
            nc.sync.dma_start(out=t3o[:, :, E:2 * E], in_=ones_ap)
            ps = psA.tile([128, F], f32, tag="bank", name=f"p1v_{mk}", padded_shape=[128, QB])
            for kd in range(ND):
                nc.tensor.matmul(
                    ps[:, :],
                    lhsT=xt_t[kd][:, mk * 128:(mk + 1) * 128],
                    rhs=wv_t[kd][:, :],
                    start=(kd == 0), stop=(kd == ND - 1),
                )
            # psum cols: (pair, hh, e); dest pair block: [V_a | ones | V_b]
            ps3 = ps.rearrange("p (pair hh e) -> p pair hh e", hh=2, e=E)
            bv3 = bv_bc.rearrange("p (pair hh e) -> p pair hh e", hh=2, e=E)
            t3 = t[:].rearrange("p (pair c) -> p pair c", c=192)
            nc.vector.tensor_tensor(
                t3[:, :, 0:E], ps3[:, :, 0, :], bv3[:, :, 0, :], op=ALU.add)
            nc.vector.tensor_tensor(
                t3[:, :, 2 * E:3 * E], ps3[:, :, 1, :], bv3[:, :, 1, :], op=ALU.add)
            v_t.append(t)

        # --- phase 2: attention ---
        ctxt_t = [big.tile([128, S], f32r, tag="big", name=f"ctxt{m}") for m in range(NF)]
        for hp in range(NF):          # head pair (partitions 0:64 / 64:128)
            for nq in range(NQB):
                cps = [
                    psA.tile([128, QB], f32, tag="bank", name=f"c{hp}_{nq}_{hh}")
                    for hh in range(2)
                ]
                for g in range(NK // KG):
                    for hh in range(2):
                        sps = pssc.tile([128, KG * QB], f32, tag="sc",
                                        name=f"s{hp}_{nq}_{g}_{hh}")
                        for j in range(KG):
                            mk = g * KG + j
                            nc.tensor.matmul(
                                sps[:, j * QB:(j + 1) * QB],
                                lhsT=kt_t[hp][:, mk * 128:(mk + 1) * 128],
                                rhs=qt_t[2 * hp + hh][:, nq * QB:(nq + 1) * QB],
                                start=True, stop=True,
                            )
                        esb = epool.tile([128, KG * QB], f32r, tag="e",
                                         name=f"e{hp}_{nq}_{g}_{hh}")
                        nc.scalar.activation(esb[:], sps[:], AF.Exp, scale=0.125)
                        for j in range(KG):
                            mk = g * KG + j
                            base = hp * 192 + hh * E
                            nc.tensor.matmul(
                                cps[hh][:, :],
                                lhsT=v_t[mk][:, base:base + 128],
                                rhs=esb[:, j * QB:(j + 1) * QB],
                                start=(g == 0 and j == 0),
                                stop=(g == NK // KG - 1 and j == KG - 1),
                            )
                for hh in range(2):
                    # head a: ctx rows 0:64, sums 64:128; head b swapped
                    ctx_sl = slice(0, E) if hh == 0 else slice(E, 128)
                    sum_sl = slice(E, 128) if hh == 0 else slice(0, E)
                    r = npool.tile([E, QB], f32, tag="n", name=f"n{hp}_{nq}_{hh}")
                    nc.vector.reciprocal(r[:], cps[hh][sum_sl, :])
                    nc.vector.tensor_tensor(
                        ctxt_t[hp][hh * E:(hh + 1) * E, nq * QB:(nq + 1) * QB],
                        cps[hh][ctx_sl, :], r[:], op=ALU.mult,
                    )

        # --- phase 3: partial output projection [S, D] ---
        for mq in range(NK):
            ps = pssc.tile([128, D], f32, tag="sc", name=f"o{mq}")
            for kf in range(NF):
                lhsT = ctxt_t[kf][:, mq * 128:(mq + 1) * 128]
                nc.tensor.matmul(ps[:, 0:512], lhsT=lhsT,
                                 rhs=wo_t[kf][:, 0:512],
                                 start=(kf == 0), stop=(kf == NF - 1))
                nc.tensor.matmul(ps[:, 512:D], lhsT=lhsT,
                                 rhs=wo_t[kf][:, 512:D],
                                 start=(kf == 0), stop=(kf == NF - 1))
            osb = opool.tile([128, D], f32, tag="o", name=f"ot{mq}")
            nc.vector.tensor_copy(osb[:, 0:512], ps[:, 0:512])
            nc.vector.tensor_copy(osb[:, 512:D], ps[:, 512:D])
            nc.sync.dma_start(out=y_d[mq * 128:(mq + 1) * 128, :], in_=osb[:])
    nc.compile()
    return nc


def _get_nc():
    global _NC
    if _NC is None:
        _NC = _build()
    return _NC


def kernel(x, Wq, bq, Wk, bk, Wv, bv, Wo, bo, _trace=False):
    x = np.asarray(x, dtype=np.float32)
    Wq = np.asarray(Wq, dtype=np.float32)
    bq = np.asarray(bq, dtype=np.float32)
    Wk = np.asarray(Wk, dtype=np.float32)
    bk = np.asarray(bk, dtype=np.float32)
    Wv = np.asarray(Wv, dtype=np.float32)
    bv = np.asarray(bv, dtype=np.float32)
    Wo = np.asarray(Wo, dtype=np.float32)
    bo = np.asarray(bo, dtype=np.float32)

    nc = _get_nc()
    in_maps = []
    for c in range(NCORES):
        b = c // 2
        h0 = (c % 2) * HL
        in_maps.append({
            "xt": np.ascontiguousarray(x[b].T),
            "wq": np.ascontiguousarray(Wq[h0:h0 + HL].transpose(1, 0, 2).reshape(D, F)),
            "wk": np.ascontiguousarray(Wk[h0:h0 + HL].transpose(1, 0, 2).reshape(D, F)),
            "wv": np.ascontiguousarray(Wv[h0:h0 + HL].transpose(1, 0, 2).reshape(D, F)),
            "wo": np.ascontiguousarray(Wo[h0 * E:(h0 + HL) * E]),
            "bq": np.ascontiguousarray(bq[h0:h0 + HL].reshape(F, 1)),
            "bk": np.ascontiguousarray(bk[h0:h0 + HL].reshape(F, 1)),
            "bv": np.ascontiguousarray(bv[h0:h0 + HL].reshape(1, F)),
            "ones": np.ones((1, NF * E), np.float32),
        })
    res = run_bass_kernel_spmd(nc, in_maps, list(range(NCORES)), trace=_trace)
    out = np.empty((B, S, D), np.float32)
    for b in range(B):
        out[b] = res.results[2 * b]["y"] + res.results[2 * b + 1]["y"] + bo[None, :]
    if _trace:
        kernel.last_exec_time_ns = res.exec_time_ns
        kernel.last_results = res
    return out
